# revision 20
# baseline (speedup 1.0000x reference)
"""Trainium2 Bass kernel for a quantized KAN layer (B-spline MLP).

  out[b,o] = x @ base_weight.T + einsum('bic,oic->bo', bspline_basis(x), round(32*w)/32)

Strategy (8 NeuronCores, contraction/i-sharded), v4:
  - Cubic B-splines on a uniform grid reproduce constants and linears
    exactly on [-1, 1]:  sum_c B_c(x) = 1  and  sum_c gamma_c B_c(x) = x
    with Greville abscissae gamma_c = (c-1)*h - 1. So the base matmul
    folds into the spline weights (v_c = q_c + gamma_c*bw) and channel 7
    folds into a per-output bias (w''_c = v_c - v_7, bias_o = sum_i v_7).
    The contraction shrinks from 9 to 7 channels: 14 k-tiles per core.
  - All input prep runs on the HOST: quantize + fold in f32 (bit-exact
    RNE), weights packed fp16 [128, 14, 2048] per core, and the 7
    B-spline basis channels evaluated in f32 closed form
    (bell(t) = ((2-|t|)^3+ - 4(1-|t|)^3+)/6) and shipped fp16, packed
    per (i-tile, batch-chunk) so each chunk's basis is one contiguous
    ~0.9MB DMA. HW ablation showed the on-device basis chain (DVE+ACT)
    slowed the matmul stream from ~217 to ~274 ns/MM via SBUF
    contention; pure DMA'd basis keeps the PE at its streaming rate.
    The einsum (99.6% of FLOPs) stays on device.
  - Weights stream in matmul consumption order (ob-quarter outer,
    k-half blocks) across both HWDGE queues (SP + ACT); chunk-0 basis
    is prefetched ahead of them so the first matmuls start ~15us in.
  - Output is computed transposed ([out, batch]) so bias_o is a
    per-partition bias applied for free in the ACT PSUM->SBUF drain;
    partials ship bf16 (halves the output stream; ~1e-3 quadrature
    error, final 8-way sum on host in f32).
  - Matmuls are fp16 x fp16 -> f32 PSUM, one 512-col moving matmul per
    stationary, k-inner order, 8 PSUM banks in flight; the loop is
    PE-streaming-bound at ~217 ns/MM (measured pure-MM rate).
  - Output DMAs alternate across both HWDGE queues.
"""

import numpy as np

B, IN, OUT = 4096, 2048, 2048
NCORES = 8
ISH = IN // NCORES          # 256 input features per core
P = 128
NT = ISH // P               # 2 i-tiles per core
NCH = 8                     # spline channels in the reference
NCH7 = 7                    # folded channels on device
KT = NT * NCH7              # 14 k-tiles
BCH = 512                   # batch chunk
NBC = B // BCH              # 8
NOB = OUT // P              # 16 output blocks
CW = NCH7 * BCH             # per-(tile, chunk) basis block columns

_BUILT = {}


def _build(h, repeat=1, wrep=1):
    from concourse import bacc, bass, mybir, tile

    f32 = mybir.dt.float32
    fp16 = mybir.dt.float16
    bf16 = mybir.dt.bfloat16
    AF = mybir.ActivationFunctionType

    nc = bacc.Bacc("TRN2", target_bir_lowering=False, debug=False)

    # Host-evaluated basis: row i (= t*128+p), col = bc*CW + c*BCH + j.
    basd = nc.dram_tensor("basd", [ISH, NBC * CW], fp16, kind="ExternalInput")
    # Host-folded fp16 weights: col = oq*(KT*OQ) + k*OQ + o_local.
    wfh = nc.dram_tensor("wfh", [P, KT * OUT], fp16, kind="ExternalInput")
    # Per-output bias, laid out [p][ob]
    biasd = nc.dram_tensor("biasd", [P, NOB], f32, kind="ExternalInput")
    outp = nc.dram_tensor("outp", [OUT, B], bf16, kind="ExternalOutput")

    with tile.TileContext(nc) as tc:
        with (
            tc.tile_pool(name="const", bufs=1) as cpool,
            tc.tile_pool(name="bas", bufs=4) as bpool,
            tc.tile_pool(name="outsb", bufs=4) as opool,
            tc.tile_pool(name="psum", bufs=8,
                         space=bass.MemorySpace.PSUM) as ppool,
        ):
            sh3 = [P, NCH7, BCH]

            def bas_dma(bt, t, bc):
                eng = nc.sync if t % 2 == 0 else nc.scalar
                eng.dma_start(
                    bt[:], basd[t * P:(t + 1) * P,
                                bc * CW:(bc + 1) * CW])

            # Prefetch chunk-0 basis ahead of the weight stream.
            pref = {}
            for t in range(NT):
                bt_ = bpool.tile(sh3, fp16, tag=f"bas{t}")
                bas_dma(bt_, t, 0)
                pref[t] = bt_

            # Bias first on ACT (tiny): needed by the first PSUM drain.
            bias_sb = cpool.tile([P, NOB], f32)
            nc.scalar.dma_start(bias_sb[:], biasd[:, :])
            # Resident folded weights, streamed in matmul consumption
            # order (ob-quarter outer), each quarter's two k-halves in
            # parallel on the two HWDGE queues.
            wf = cpool.tile([P, KT, OUT], fp16)
            OQ = OUT // 4
            KH = KT // 2
            for r in range(wrep):
                for oq in range(4):
                    for half, eng in ((0, nc.scalar), (1, nc.sync)):
                        k0, k1 = half * KH, (half + 1) * KH
                        eng.dma_start(
                            wf[:, k0:k1, oq * OQ:(oq + 1) * OQ],
                            wfh[:, (oq * KT + k0) * OQ:(oq * KT + k1) * OQ])

            # ---- main loop ----
            first = True
            for bc in [c for _ in range(repeat) for c in range(NBC)]:
                bas = []
                for t in range(NT):
                    if first and bc == 0:
                        bas.append(pref[t])
                        continue
                    bt_ = bpool.tile(sh3, fp16, tag=f"bas{t}")
                    bas_dma(bt_, t, bc)
                    bas.append(bt_)
                first = False

                for ob in range(NOB):
                    ps = ppool.tile([P, BCH], f32, tag="ps")
                    k = 0
                    for t in range(NT):
                        for c in range(NCH7):
                            nc.tensor.matmul(
                                ps[:],
                                wf[:, t * NCH7 + c, ob * P:(ob + 1) * P],
                                bas[t][:, c, :],
                                start=(k == 0), stop=(k == KT - 1))
                            k += 1
                    osb = opool.tile([P, BCH], bf16, tag="osb")
                    nc.scalar.activation(osb[:], ps[:], AF.Identity,
                                         bias=bias_sb[:, ob:ob + 1],
                                         scale=1.0)
                    # Alternate output DMAs across both HWDGE queues.
                    eng = nc.sync if ob % 2 == 0 else nc.scalar
                    eng.dma_start(
                        outp[ob * P:(ob + 1) * P,
                             bc * BCH:(bc + 1) * BCH], osb[:])

    nc.compile()
    return nc


def _stage(x, base_weight, spline_weight, grid):
    """Per-core host staging in f32 (bit-exact RNE quantize + fold; basis in
    closed form), packed fp16 for the device."""
    h = np.float32(grid[0, 1] - grid[0, 0])
    gam7 = np.float32((NCH - 2) * h - 1.0)
    inv_h = np.float32(1.0) / h
    in_maps = []
    # round(32w)/32 in f32, RNE — matches the reference quantizer.
    q_all = (np.round(spline_weight.astype(np.float32) * np.float32(32.0))
             * np.float32(1.0 / 32.0)).astype(np.float32)
    x32 = x.astype(np.float32)
    for j in range(NCORES):
        sh = slice(j * ISH, (j + 1) * ISH)
        # Basis: B_c(x) = bell((x+1)/h + 1 - c), c = 0..6, evaluated f32.
        u = (x32[:, sh].T * inv_h + (inv_h + np.float32(1.0)))  # [ISH, B]
        basc = np.empty((ISH, NCH7, B), dtype=np.float32)
        for c in range(NCH7):
            t = np.abs(u - np.float32(c))
            a = np.maximum(np.float32(2.0) - t, np.float32(0.0))
            bcb = np.maximum(np.float32(1.0) - t, np.float32(0.0))
            basc[:, c, :] = (a * a * a - np.float32(4.0) * bcb * bcb * bcb) \
                * np.float32(1.0 / 6.0)
        # pack [i, bc*CW + c*BCH + j]
        basd = basc.reshape(ISH, NCH7, NBC, BCH).transpose(0, 2, 1, 3)
        basd = np.ascontiguousarray(
            basd.reshape(ISH, NBC * CW).astype(np.float16))

        q = q_all[:, sh, :]                         # [OUT, 256, 8] f32
        bw = base_weight[:, sh].astype(np.float32)  # [OUT, 256]
        q7 = q[:, :, NCH - 1]
        v7 = q7 + gam7 * bw
        bias = v7.sum(axis=1, dtype=np.float32)
        wfold = np.empty((OUT, ISH, NCH7), dtype=np.float32)
        for c in range(NCH7):
            wfold[:, :, c] = (q[:, :, c] - q7) + np.float32((c - 7) * h) * bw
        # layout [p, t*7+c, o], then regroup columns as [oq][k][o_local]
        wfh = wfold.reshape(OUT, NT, P, NCH7).transpose(2, 1, 3, 0)
        wfh = wfh.reshape(P, KT, 4, OUT // 4).transpose(0, 2, 1, 3)
        wfh = np.ascontiguousarray(
            wfh.reshape(P, KT * OUT).astype(np.float16))
        biasd = np.ascontiguousarray(bias.reshape(NOB, P).T)
        in_maps.append({"basd": basd, "wfh": wfh, "biasd": biasd})
    return in_maps


def kernel(x, base_weight, spline_weight, grid, _profile=None):
    from concourse import bass_utils

    x = np.asarray(x, dtype=np.float32)
    base_weight = np.asarray(base_weight, dtype=np.float32)
    spline_weight = np.asarray(spline_weight, dtype=np.float32)
    grid = np.asarray(grid, dtype=np.float32)

    h = float(grid[0, 1] - grid[0, 0])
    key = round(h, 9)
    if key not in _BUILT:
        _BUILT[key] = _build(h)
    nc = _BUILT[key]

    in_maps = _stage(x, base_weight, spline_weight, grid)
    kw = {}
    if _profile is not None:
        kw = _profile
    res = bass_utils.run_bass_kernel_spmd(
        nc, in_maps, core_ids=list(range(NCORES)), **kw)

    out_T = np.zeros((OUT, B), dtype=np.float32)
    for om in res.results:
        out_T += np.asarray(om["outp"], dtype=np.float32)
    if _profile is not None:
        kernel._last_result = res
    return np.ascontiguousarray(out_T.T)
